# revision 29
# baseline (speedup 1.0000x reference)
"""Trainium2 Bass kernel for nn_CrossMambaBlock.

kernel(**inputs) takes the FULL inputs from setup_inputs() (x: (8,1024,256)
fp32 + nested params) and returns the FULL (8,1024,256) output.  Sharding:
data-parallel over batch across 8 NeuronCores (one batch element per core,
params replicated), one SPMD Bass program via run_bass_kernel_spmd.

Layout conventions per core:
  FM (feature-major): SBUF tile [128, n_ch_tiles, L]   channel c = t*128+p
  TM (token-major):   SBUF tile [128, n_tok_tiles, C]  token  l = i*128+p
All matmuls run in bf16 (full PE speed; fp32 would be 4x slower and
float32r fails BIR verification for non-rounded producers); the SSM/
attention tensors are bf16 with fp32 PSUM accumulation throughout.
The SSM recurrence uses DVE tensor_tensor_scan (fp32 internal
state); the backward direction scans through reversed APs.  dt=softplus is
a fused Square(s*x+b)+r quadratic (softplus has no ACT table on TRN2); the
depthwise conv is folded into the in_proj matmuls via pre-scaled weights
and a shifted rhs access pattern; B/C state vectors are broadcast across
partitions with a DRAM round-trip broadcast DMA.  Small parameter vectors
and weight matrices are packed into shared SBUF tiles (tiles pad to 4KB
per partition, so one tile per vector would blow the budget).
"""

import numpy as np

DIM = 256
DSTATE = 8
DTRANK = 16
HEADS = 4
HD = 64
MANCH = 512
NTOK = 1024
NCORES = 8
P = 128

_prog_cache = {}

_DIRS = ('b0f', 'b0b', 'b1f', 'b1b')
_CBR = ('q', 'k', 'v0', 'v1', 'mh0', 'mh1')


def _vec_layout():
    """Column layout of the packed per-partition vector tile."""
    off = {}
    o = 0
    for d in _DIRS:
        for nm, w in ((f'{d}_convb', 2), (f'{d}_sqbias', 2), (f'{d}_Asc', 16),
                      (f'{d}_Abias', 16), (f'{d}_D', 2)):
            off[nm] = o
            o += w
    off['catvec0'] = o
    o += 2
    for nm in _CBR:
        off[f'{nm}_s'] = o
        o += 2
        off[f'{nm}_t'] = o
        o += 2
    off['eps'] = o
    o += 2
    return off, o


# ---------------------------------------------------------------- host prep

def _softplus_quadfit(lo, hi):
    xs = np.linspace(lo, hi, 4001).astype(np.float64)
    c2, c1, c0 = np.polyfit(xs, np.log1p(np.exp(xs)), 2)
    s = np.sqrt(c2)
    b = c1 / (2 * s)
    r = c0 - b * b
    err = np.abs((s * xs + b) ** 2 + r - np.log1p(np.exp(xs))).max()
    return float(s), float(b), float(r), float(err)


def _dtraw_range(x, p):
    xs = x[: min(2, x.shape[0])].astype(np.float64)

    def ln_(v, w, b, eps=1e-5):
        m = v.mean(-1, keepdims=True)
        var = v.var(-1, keepdims=True)
        return (v - m) / np.sqrt(var + eps) * w + b

    def silu(v):
        return v / (1 + np.exp(-v))

    x0 = ln_(xs, p['ln0_w'], p['ln0_b'])
    E = np.exp(x0 @ p['map_w'].T.astype(np.float64))
    s_map = E / E.sum(1, keepdims=True)
    x1 = np.einsum('bnc,bnm->bmc', x0, s_map)
    lo, hi = [], []
    for bp, u in ((p['block0'], x0), (p['block1'], x1)):
        xz = u @ bp['in_proj_w'].T.astype(np.float64)
        xp_ = xz[..., :DIM]
        for dname in ('fwd', 'bwd'):
            pr = bp[dname]
            xin = xp_[:, ::-1] if dname == 'bwd' else xp_
            xprev = np.concatenate([np.zeros_like(xin[:, :1]), xin[:, :-1]], 1)
            xc = silu(xprev * pr['conv_w'][:, 0] + xin * pr['conv_w'][:, 1]
                      + pr['conv_b'])
            dtr = ((xc @ pr['x_proj_w'].T.astype(np.float64))[..., :DTRANK]
                   @ pr['dt_proj_w'].T.astype(np.float64) + pr['dt_proj_b'])
            lo.append(dtr.min())
            hi.append(dtr.max())
    return float(min(lo)), float(max(hi))


def _prep(x, params):
    import ml_dtypes
    bf16 = ml_dtypes.bfloat16
    f32 = np.float32
    f64 = np.float64
    p = params
    cm = {}

    lo, hi = _dtraw_range(x, p)
    s, b, r, ferr = _softplus_quadfit(lo - 0.3, hi + 0.3)
    assert ferr < 2e-3, f"softplus quad fit err {ferr} over [{lo},{hi}]"

    cm['ident_bf'] = np.eye(P, dtype=bf16)
    cm['ones_bf'] = np.ones((P, 1), dtype=bf16)

    VO, NV = _vec_layout()
    vecs = np.zeros((NV, P), f64)

    def setv(name, v):
        v = np.asarray(v, f64).reshape(-1, P)
        vecs[VO[name]:VO[name] + v.shape[0]] = v

    bigsq = {0: [], 1: []}
    wconv = {0: [], 1: []}
    xp_pack = {0: [], 1: []}
    dtp_pack = {0: [], 1: []}
    for bi in (0, 1):
        bp = p[f'block{bi}']
        W_in = np.asarray(bp['in_proj_w'], f64)
        Wx, Wz = W_in[:DIM], W_in[DIM:]
        for dn in ('fwd', 'bwd'):
            pr = bp[dn]
            d = f'b{bi}{dn[0]}'
            w0 = np.asarray(pr['conv_w'], f64)[:, 0:1]
            w1 = np.asarray(pr['conv_w'], f64)[:, 1:2]
            wconv[bi] += [(w1 * Wx).T, (w0 * Wx).T]
            setv(f'{d}_convb', pr['conv_b'])
            xpT = np.asarray(pr['x_proj_w'], f64).T
            xp_pack[bi].append(np.concatenate(
                [xpT[:, :DTRANK], np.zeros((DIM, 32 - DTRANK)),
                 xpT[:, DTRANK:]], axis=1))   # dt | pad | B C (32-aligned)
            dtp_pack[bi].append(np.asarray(pr['dt_proj_w'], f64).T)
            setv(f'{d}_sqbias', s * np.asarray(pr['dt_proj_b'], f64) + b)
            A = -np.exp(np.asarray(pr['A_log'], f64))
            setv(f'{d}_Asc', A.T.reshape(-1))
            setv(f'{d}_Abias', (A * r).T.reshape(-1))
            setv(f'{d}_D', pr['D'])
        Wo = np.asarray(bp['out_proj_w'], f64)
        WoT = (Wo * np.asarray(bp['norm_w'], f64)[None, :]).T
        ob = Wo @ np.asarray(bp['norm_b'], f64)
        Wcat = np.asarray(p[f'cat{bi}_w'], f64)
        catvec = Wcat @ ob
        bigsq[bi] += [Wz.T, WoT, Wcat.T]
        if bi == 0:
            setv('catvec0', catvec)
        else:
            catvec1 = catvec
    bigsq[0].append(0.5 * np.asarray(p['proj_w'], f64).T)

    cbrw = []
    for nm in _CBR:
        cp = p['cross'][nm]
        sc = np.asarray(cp['gamma'], f64) / np.sqrt(np.asarray(cp['var'], f64)
                                                    + 1e-5)
        tc_ = (np.asarray(cp['b'], f64) - np.asarray(cp['mean'], f64)) * sc \
            + np.asarray(cp['beta'], f64)
        if nm == 'q':
            sc = sc / np.sqrt(HD)
            tc_ = tc_ / np.sqrt(HD)
        cbrw.append(np.asarray(cp['w'], f64).T)
        setv(f'{nm}_s', sc)
        setv(f'{nm}_t', tc_)
    setv('eps', np.concatenate([np.full(P, 1e-5), np.full(P, 4e-5)]))

    cm['vecs'] = vecs.astype(f32)
    cm['bigsq0'] = np.ascontiguousarray(np.concatenate(bigsq[0], 1)).astype(bf16)
    cm['bigsq1'] = np.ascontiguousarray(np.concatenate(bigsq[1], 1)).astype(bf16)
    cm['b0_wconv'] = np.ascontiguousarray(np.concatenate(wconv[0], 1)).astype(bf16)
    cm['b1_wconv'] = np.ascontiguousarray(np.concatenate(wconv[1], 1)).astype(bf16)
    cm['b0_xp'] = np.ascontiguousarray(np.concatenate(xp_pack[0], 1)).astype(bf16)
    cm['b1_xp'] = np.ascontiguousarray(np.concatenate(xp_pack[1], 1)).astype(bf16)
    cm['b0_dtp'] = np.ascontiguousarray(np.concatenate(dtp_pack[0], 1)).astype(bf16)
    cm['b1_dtp'] = np.ascontiguousarray(np.concatenate(dtp_pack[1], 1)).astype(bf16)
    cm['cbrw'] = np.ascontiguousarray(np.concatenate(cbrw, 1)).astype(bf16)
    cm['map_T'] = np.ascontiguousarray(np.asarray(p['map_w'], f64).T).astype(bf16)
    cm['unmap_T'] = np.ascontiguousarray(np.asarray(p['unmap_w'], f64).T).astype(bf16)
    cm['rowvecs'] = np.concatenate(
        [np.asarray(p['ln0_w'], f64).reshape(1, DIM),
         np.asarray(p['ln0_b'], f64).reshape(1, DIM),
         catvec1.reshape(1, DIM)], 1).astype(f32)
    return cm, dict(sq_s=s, sq_r=r)


# ---------------------------------------------------------------- builder

def _patch_sim_silu():
    """CoreSim lacks Silu (HW has it via the silu_and_others ACT table).
    Run the Sigmoid path, then multiply the output in place by the
    scaled-and-biased input.  Simulation-only; the HW NEFF is untouched."""
    import concourse.bass_interp as bi
    import concourse.mybir as mb
    if getattr(bi, '_silu_patched', False):
        return
    orig = bi.InstructionExecutor.visit_InstActivation
    Direction = bi.Direction

    def visit(self, instruction, reg_snapshot=None):
        if instruction.func != mb.ActivationFunctionType.Silu:
            return orig(self, instruction, reg_snapshot=reg_snapshot)
        instruction.func = mb.ActivationFunctionType.Sigmoid
        try:
            res = orig(self, instruction, reg_snapshot=reg_snapshot)
        finally:
            instruction.func = mb.ActivationFunctionType.Silu
        ins = instruction.ins
        inp = self.view_ap(ins[0], Direction.READ, instruction,
                           reg_snapshot=reg_snapshot).astype(np.float32)
        bv = (ins[1].value if isinstance(ins[1], mb.ImmediateValue)
              else self.view_ap(ins[1], Direction.READ, instruction,
                                reg_snapshot=reg_snapshot).astype(np.float32))
        sv = (ins[2].value if isinstance(ins[2], mb.ImmediateValue)
              else self.view_ap(ins[2], Direction.READ, instruction,
                                reg_snapshot=reg_snapshot).astype(np.float32))
        inp = inp.reshape(inp.shape[0], -1)
        if hasattr(bv, 'reshape'):
            bv = bv.reshape(bv.shape[0], -1)
        if hasattr(sv, 'reshape'):
            sv = sv.reshape(sv.shape[0], -1)
        u = inp * sv + bv
        outv = self.view_ap(instruction.outs[0], Direction.WRITE, instruction,
                            reg_snapshot=reg_snapshot)
        flat = outv.reshape(outv.shape[0], -1)
        flat[...] = (flat.astype(np.float32) * u).astype(flat.dtype)
        return res

    bi.InstructionExecutor.visit_InstActivation = visit
    bi._silu_patched = True


def _build(consts):
    import concourse.bacc as bacc
    import concourse.mybir as mybir
    import concourse.tile as tile
    from contextlib import ExitStack

    dt = mybir.dt
    F32, BF16, F32R = dt.float32, dt.bfloat16, dt.float32r
    AT = mybir.ActivationFunctionType
    OP = mybir.AluOpType
    AX = mybir.AxisListType

    _patch_sim_silu()
    nc = bacc.Bacc("TRN2", target_bir_lowering=False, debug=False)
    sq_s, sq_r = consts['sq_s'], consts['sq_r']
    NT, MT = NTOK // P, MANCH // P      # 8, 4
    VO, NV = _vec_layout()

    def din(name, shape, dtype=F32):
        return nc.dram_tensor(name, shape, dtype, kind="ExternalInput").ap()

    SHAPES = {
        'ident_bf': ((P, P), BF16),
        'ones_bf': ((P, 1), BF16), 'vecs': ((NV, P), F32),
        'bigsq0': ((DIM, 4 * DIM), BF16), 'bigsq1': ((DIM, 3 * DIM), BF16),
        'b0_wconv': ((DIM, 4 * DIM), BF16), 'b1_wconv': ((DIM, 4 * DIM), BF16),
        'b0_xp': ((DIM, 96), BF16), 'b1_xp': ((DIM, 96), BF16),
        'b0_dtp': ((DTRANK, 2 * DIM), BF16), 'b1_dtp': ((DTRANK, 2 * DIM), BF16),
        'cbrw': ((DIM, 6 * DIM), BF16),
        'map_T': ((DIM, MANCH), BF16), 'unmap_T': ((DIM, MANCH), BF16),
        'rowvecs': ((1, 3 * DIM), F32),
    }
    dr = {n: din(n, *SHAPES[n]) for n in SHAPES}
    x_d = din('x', (NTOK, DIM))
    xb_d = din('xb', (NTOK, DIM))
    out_d = nc.dram_tensor('out', (NTOK, DIM), F32, kind="ExternalOutput").ap()

    bc_stage = {}
    for bi, L in ((0, NTOK), (1, MANCH)):
        for dn in 'fb':
            bc_stage[f'b{bi}{dn}'] = nc.dram_tensor(
                f'bcst_b{bi}{dn}', (2 * DSTATE, L), BF16, kind="Internal").ap()
    r1_stage = nc.dram_tensor('r1_stage', (1, NTOK), BF16, kind="Internal").ap()

    with tile.TileContext(nc) as tc, ExitStack() as top:
        wp = top.enter_context(tc.tile_pool(name="wp", bufs=1))
        gp = top.enter_context(tc.tile_pool(name="gp", bufs=1))
        pA = top.enter_context(tc.tile_pool(name="pA", bufs=3, space="PSUM"))
        pT = top.enter_context(tc.tile_pool(name="pT", bufs=2, space="PSUM"))
        pY = top.enter_context(tc.tile_pool(name="pY", bufs=2, space="PSUM"))

        def psA():
            return pA.tile([P, 512], F32, tag="ps", name="psA")

        def psT(dty=F32):
            return pT.tile([P, 4 * P], dty, tag="ps", name="psT")

        W = {}

        def load_w(n, pool):
            shp, dty = SHAPES[n]
            if n == 'vecs':
                t = pool.tile([P, NV], dty, tag=n, name=n)
                nc.sync.dma_start(t[:], dr[n].transpose([1, 0]))
            elif shp[0] > P:
                k = shp[0] // P
                t = pool.tile([P, k, shp[1]], dty, tag=n, name=n)
                nc.sync.dma_start(t[:], dr[n].rearrange("(k p) m -> p k m", p=P))
            else:
                t = pool.tile(list(shp), dty, tag=n, name=n)
                nc.sync.dma_start(t[:], dr[n])
            W[n] = t

        for n in ('ident_bf', 'ones_bf', 'vecs',
                  'bigsq0', 'bigsq1', 'b0_wconv', 'b1_wconv',
                  'b0_xp', 'b1_xp', 'b0_dtp', 'b1_dtp', 'rowvecs'):
            load_w(n, wp)
        assert set(SHAPES) - set(W) == {'map_T', 'unmap_T', 'cbrw'}, set(SHAPES) - set(W)
        IDb = W['ident_bf']

        def V(name, j=0):
            return W['vecs'][:, VO[name] + j:VO[name] + j + 1]

        def bcast_row(dst, row_ap):
            nc.sync.dma_start(dst, row_ap.partition_broadcast(P))

        row_bc = gp.tile([P, 3 * DIM], F32, tag="row_bc")
        bcast_row(row_bc[:], dr['rowvecs'])
        ln0w_bc = row_bc[:, 0:DIM]
        ln0b_bc = row_bc[:, DIM:2 * DIM]
        catvec1_bc = row_bc[:, 2 * DIM:3 * DIM]

        # persistent activations
        x0_tm_bf = gp.tile([P, NT, DIM], BF16, tag="x0_tm_bf")
        x0_fm = gp.tile([P, 2, NTOK], BF16, tag="x0_fm")
        x1_fm = gp.tile([P, 2, MANCH], BF16, tag="x1_fm")
        x1_tm = gp.tile([P, MT, DIM], F32, tag="x1_tm")
        out0_fm = gp.tile([P, 2, NTOK], BF16, tag="out0_fm")
        out1_tm = gp.tile([P, MT, DIM], BF16, tag="out1_tm")

        def to_fm(src_tm, dst_fm, n_tok_tiles, bf=True):
            for t in range(2):
                for i0 in range(0, n_tok_tiles, 4):
                    nb = min(4, n_tok_tiles - i0)
                    ps = psT(BF16)
                    for j in range(nb):
                        i = i0 + j
                        nc.tensor.transpose(ps[:, j * P:(j + 1) * P],
                                            src_tm[:, i, t * P:(t + 1) * P],
                                            IDb[:])
                    nc.scalar.copy(dst_fm[:, t, i0 * P:(i0 + nb) * P],
                                   ps[:, :nb * P])

        def to_tm(src_fm, dst_tm, n_tok_tiles, bf=True, evict=None):
            for i in range(n_tok_tiles):
                ps = psT(BF16)
                for t in range(2):
                    nc.tensor.transpose(ps[:, t * P:(t + 1) * P],
                                        src_fm[:, t, i * P:(i + 1) * P],
                                        IDb[:])
                if evict is None:
                    nc.scalar.copy(dst_tm[:, i, :], ps[:, :2 * P])
                else:
                    evict(i, ps[:, :2 * P])

        def mm(ps_ap, pairs, bf=True):
            n = len(pairs)
            for i, (lt, rh) in enumerate(pairs):
                nc.tensor.matmul(ps_ap, lt, rh, start=(i == 0), stop=(i == n - 1))

        # ================= LN0 (token-major) =================
        with tc.tile_pool(name="ln0", bufs=1) as lp:
            x0_tm = lp.tile([P, NT, DIM], F32, tag="x0_tm")
            x_tm = lp.tile([P, NT, DIM], F32, tag="x_tm")
            nc.sync.dma_start(x_tm[:], x_d.rearrange("(i p) c -> p i c", p=P))
            st = lp.tile([P, 6 * NT], F32, tag="st")
            sums, sumsq = st[:, 0:NT], st[:, NT:2 * NT]
            mean, rstd = st[:, 2 * NT:3 * NT], st[:, 3 * NT:4 * NT]
            tmpv, msq = st[:, 4 * NT:5 * NT], st[:, 5 * NT:6 * NT]
            xsq = lp.tile([P, NT, DIM], BF16, tag="xsq")
            nc.vector.reduce_sum(sums, x_tm[:], axis=AX.X)
            nc.scalar.activation(xsq[:], x_tm[:], AT.Square)
            nc.vector.reduce_sum(sumsq, xsq[:], axis=AX.X)
            nc.vector.tensor_scalar_mul(mean, sums, 1.0 / DIM)
            nc.vector.tensor_scalar_mul(tmpv, sumsq, 1.0 / DIM)
            nc.vector.tensor_tensor(msq, mean, mean, OP.mult)
            nc.vector.tensor_tensor(tmpv, tmpv, msq, OP.subtract)
            nc.scalar.activation(tmpv, tmpv, AT.Sqrt, bias=V('eps', 0))
            nc.vector.reciprocal(rstd, tmpv)
            for i in range(NT):
                t1 = lp.tile([P, DIM], F32, tag="t1", name="t1")
                nc.vector.tensor_scalar(t1[:], x_tm[:, i, :], mean[:, i:i + 1],
                                        rstd[:, i:i + 1], OP.subtract, OP.mult)
                t2 = lp.tile([P, DIM], F32, tag="t2", name="t2")
                nc.vector.tensor_tensor(t2[:], t1[:], ln0w_bc, OP.mult)
                nc.vector.tensor_tensor(x0_tm[:, i, :], t2[:], ln0b_bc, OP.add)
                nc.vector.tensor_copy(x0_tm_bf[:, i, :], x0_tm[:, i, :])
            to_fm(x0_tm_bf, x0_fm, NT)

        # ================= map softmax -> x1 =================
        with tc.tile_pool(name="mapp", bufs=1) as sp:
            load_w('map_T', sp)
            Emap = sp.tile([P, MT, NTOK], BF16, tag="Emap")
            rsml = sp.tile([P, 4 * MT], F32, tag="rsml")
            rs_map = rsml[:, 0:2 * MT]
            rsum, rmap = rsml[:, 2 * MT:3 * MT], rsml[:, 3 * MT:4 * MT]
            for mt in range(MT):
                for ch in range(2):
                    ps = psA()
                    mm(ps[:], [(W['map_T'][:, kt, mt * P:(mt + 1) * P],
                                x0_fm[:, kt, ch * 512:(ch + 1) * 512])
                               for kt in range(2)])
                    nc.scalar.activation(Emap[:, mt, ch * 512:(ch + 1) * 512],
                                         ps[:], AT.Exp,
                                         accum_out=rs_map[:, mt * 2 + ch:
                                                          mt * 2 + ch + 1])
            nc.vector.reduce_sum(
                rsum, rs_map.rearrange("p (m c) -> p m c", c=2), axis=AX.X)
            nc.vector.reciprocal(rmap, rsum)
            smap = sp.tile([P, MT, NTOK], BF16, tag="smap")
            for mt in range(MT):
                nc.scalar.activation(smap[:, mt, :], Emap[:, mt, :], AT.Copy,
                                     scale=rmap[:, mt:mt + 1])
            smapT = sp.tile([P, NT, MANCH], BF16, tag="smapT")
            for i in range(NT):
                ps = psT(BF16)
                for mt in range(MT):
                    nc.tensor.transpose(ps[:, mt * P:(mt + 1) * P],
                                        smap[:, mt, i * P:(i + 1) * P], IDb[:])
                nc.scalar.copy(smapT[:, i, :], ps[:, :4 * P])
            for ct in range(2):
                ps = psA()
                mm(ps[:], [(x0_tm_bf[:, i, ct * P:(ct + 1) * P], smapT[:, i, :])
                           for i in range(NT)], bf=True)
                nc.scalar.copy(x1_fm[:, ct, :], ps[:])
        to_tm(x1_fm, x1_tm, MT)

        # ================= mamba blocks =================
        def mamba(bi, u_fm, L, res, out_tile, out_is_tm):
            LT = L // P
            NCH = L // 512
            WCV = W[f'b{bi}_wconv']
            BSQ = W[f'bigsq{bi}']
            with ExitStack() as es:
                mp = es.enter_context(tc.tile_pool(name=f"m{bi}", bufs=1))
                sp_ = es.enter_context(tc.tile_pool(name=f"m{bi}s", bufs=3))
                hp = es.enter_context(tc.tile_pool(name=f"m{bi}h", bufs=1))

                # z half + gate (TM)
                z_bf = mp.tile([P, 2, L], BF16, tag="z_bf", name="z_bf")
                for t in range(2):
                    for ch in range(NCH):
                        ps = psA()
                        mm(ps[:], [(BSQ[:, kt, t * P:(t + 1) * P],
                                    u_fm[:, kt, ch * 512:(ch + 1) * 512])
                                   for kt in range(2)])
                        nc.scalar.copy(z_bf[:, t, ch * 512:(ch + 1) * 512], ps[:])
                g_tm = mp.tile([P, LT, DIM], BF16, tag="g_tm")

                def g_evict(i, ps_ap):
                    nc.scalar.activation(g_tm[:, i, :], ps_ap, AT.Silu)
                to_tm(z_bf, None, LT, bf=True, evict=g_evict)

                yg_f = mp.tile([P, LT, DIM], BF16, tag="yg_f")
                ysum = mp.tile([P, LT, DIM], F32, tag="ysum")

                for dn in 'fb':
                    d = f'b{bi}{dn}'
                    wc0 = 0 if dn == 'f' else 2 * DIM   # [W1 | W0] col base
                    # --- x-half in_proj with conv folded ---
                    xc_fm = mp.tile([P, 2, L], BF16, tag="xc_fm", name="xc_fm")
                    for t in range(2):
                        for ch in range(NCH):
                            c0, c1 = ch * 512, (ch + 1) * 512
                            ps = psA()
                            for kt in range(2):
                                nc.tensor.matmul(
                                    ps[:],
                                    WCV[:, kt, wc0 + t * P:wc0 + (t + 1) * P],
                                    u_fm[:, kt, c0:c1],
                                    start=(kt == 0), stop=False)
                            if dn == 'f':
                                lo = 1 if ch == 0 else 0
                                for kt in range(2):
                                    nc.tensor.matmul(
                                        ps[:, lo:512],
                                        WCV[:, kt, wc0 + DIM + t * P:
                                                wc0 + DIM + (t + 1) * P],
                                        u_fm[:, kt, c0 + lo - 1:c1 - 1],
                                        start=False, stop=(kt == 1))
                            else:
                                hi = 511 if ch == NCH - 1 else 512
                                for kt in range(2):
                                    nc.tensor.matmul(
                                        ps[:, 0:hi],
                                        WCV[:, kt, wc0 + DIM + t * P:
                                                wc0 + DIM + (t + 1) * P],
                                        u_fm[:, kt, c0 + 1:c0 + hi + 1],
                                        start=False, stop=(kt == 1))
                            nc.scalar.activation(xc_fm[:, t, c0:c1], ps[:],
                                                 AT.Silu,
                                                 bias=V(f'{d}_convb', t))
                    # --- x_proj ---
                    xpc = 0 if dn == 'f' else 48
                    xdbl = mp.tile([DTRANK, L], BF16, tag="z_bf",
                                   name="xdbl")
                    bc_bf = mp.tile([2 * DSTATE, L], BF16, tag="bc_bf")
                    for ch in range(NCH):
                        c0, c1 = ch * 512, (ch + 1) * 512
                        ps = psA()
                        mm(ps[:48, :],
                           [(W[f'b{bi}_xp'][:, kt, xpc:xpc + 48],
                             xc_fm[:, kt, c0:c1]) for kt in range(2)])
                        nc.scalar.copy(xdbl[:, c0:c1], ps[:DTRANK, :])
                        nc.scalar.copy(bc_bf[:, c0:c1], ps[32:48, :])
                    nc.sync.dma_start(bc_stage[d], bc_bf[:])
                    # --- dt quadratic ---
                    dtc = 0 if dn == 'f' else DIM
                    dtsq_fm = mp.tile([P, 2, L], F32, tag="dtsq_fm",
                                      name="dtsq_fm")
                    for t in range(2):
                        for ch in range(NCH):
                            c0, c1 = ch * 512, (ch + 1) * 512
                            ps = psA()
                            mm(ps[:], [(W[f'b{bi}_dtp'][:, dtc + t * P:
                                                        dtc + (t + 1) * P],
                                        xdbl[:DTRANK, c0:c1])])
                            nc.scalar.activation(dtsq_fm[:, t, c0:c1], ps[:],
                                                 AT.Square, scale=sq_s,
                                                 bias=V(f'{d}_sqbias', t))
                    du_bf = mp.tile([P, 2, L], BF16, tag="du_bf")
                    nc.vector.scalar_tensor_tensor(du_bf[:], dtsq_fm[:], sq_r,
                                                   xc_fm[:], OP.add, OP.mult)
                    xcD_bf = mp.tile([P, 2, L], BF16, tag="xcD_bf")
                    for t in range(2):
                        nc.vector.tensor_scalar_mul(xcD_bf[:, t, :],
                                                    xc_fm[:, t, :],
                                                    V(f'{d}_D', t))

                    # --- SSM: full-L scans; y-accum per channel tile ---
                    for t in range(2):
                        hC = hp.tile([P, DSTATE, L], BF16, tag="hC", name="hC")
                        for n in range(DSTATE):
                            wk = sp_.tile([P, 4, L], BF16, tag="wk", name="wk")
                            dA, dBu = wk[:, 0, :], wk[:, 1, :]
                            B_bc, C_bc = wk[:, 2, :], wk[:, 3, :]
                            nc.scalar.activation(
                                dA, dtsq_fm[:, t, :], AT.Exp,
                                scale=V(f'{d}_Asc', n * 2 + t),
                                bias=V(f'{d}_Abias', n * 2 + t))
                            bcast_row(B_bc, bc_stage[d][n:n + 1, :])
                            bcast_row(C_bc,
                                      bc_stage[d][DSTATE + n:DSTATE + n + 1, :])
                            nc.vector.tensor_tensor(dBu, du_bf[:, t, :],
                                                    B_bc, OP.mult)
                            h = sp_.tile([P, L], BF16, tag="h", name="h", bufs=2)
                            if dn == 'f':
                                nc.vector.tensor_tensor_scan(
                                    h[:], dA, dBu, 0.0, OP.mult, OP.add)
                            else:
                                nc.vector.tensor_tensor_scan(
                                    h[:, ::-1], dA[:, ::-1], dBu[:, ::-1],
                                    0.0, OP.mult, OP.add)
                            nc.vector.tensor_tensor(hC[:, n, :], h[:], C_bc,
                                                    OP.mult)
                        for i in range(LT):
                            ps = pY.tile([P, P], F32, tag="ps", name="psy")
                            for n in range(DSTATE):
                                nc.tensor.matmul(ps[:],
                                                 hC[:, n, i * P:(i + 1) * P],
                                                 IDb[:], start=(n == 0),
                                                 stop=False)
                            nc.tensor.matmul(ps[:],
                                             xcD_bf[:, t, i * P:(i + 1) * P],
                                             IDb[:], start=False, stop=True)
                            sl = (slice(None), i, slice(t * P, (t + 1) * P))
                            if dn == 'f':
                                nc.vector.tensor_tensor(yg_f[sl], ps[:],
                                                        g_tm[sl], OP.mult)
                            else:
                                tmp = sp_.tile([P, P], BF16, tag="ygb",
                                               name="ygb")
                                nc.vector.tensor_tensor(tmp[:], ps[:],
                                                        g_tm[sl], OP.mult)
                                nc.vector.tensor_tensor(ysum[sl], tmp[:],
                                                        yg_f[sl], OP.add)

                # --- LN over channels (TM); norm w/b folded into out_proj ---
                st = mp.tile([P, 6 * LT], F32, tag="st", name="st")
                s1, s2 = st[:, 0:LT], st[:, LT:2 * LT]
                mu, rs = st[:, 2 * LT:3 * LT], st[:, 3 * LT:4 * LT]
                tv, ms = st[:, 4 * LT:5 * LT], st[:, 5 * LT:6 * LT]
                ysq = mp.tile([P, LT, DIM], BF16, tag="z_bf", name="ysq")
                nc.vector.reduce_sum(s1, ysum[:], axis=AX.X)
                nc.scalar.activation(ysq[:], ysum[:], AT.Square)
                nc.vector.reduce_sum(s2, ysq[:], axis=AX.X)
                nc.vector.tensor_scalar_mul(mu, s1, 1.0 / DIM)
                nc.vector.tensor_scalar_mul(tv, s2, 1.0 / DIM)
                nc.vector.tensor_tensor(ms, mu, mu, OP.mult)
                nc.vector.tensor_tensor(tv, tv, ms, OP.subtract)
                # LN of 0.5*(yf+yb) == LN-with-4*eps of (yf+yb): exact
                nc.scalar.activation(tv, tv, AT.Sqrt, bias=V('eps', 1))
                nc.vector.reciprocal(rs, tv)
                t1_tm = mp.tile([P, LT, DIM], BF16, tag="dtsq_fm", name="t1_tm")
                for i in range(LT):
                    nc.vector.tensor_scalar(t1_tm[:, i, :], ysum[:, i, :],
                                            mu[:, i:i + 1], rs[:, i:i + 1],
                                            OP.subtract, OP.mult)
                t1_fm = mp.tile([P, 2, L], BF16, tag="xc_fm", name="t1_fm")
                to_fm(t1_tm, t1_fm, LT)
                raw_fm = mp.tile([P, 2, L], BF16, tag="du_bf", name="raw_fm")
                for t in range(2):
                    for ch in range(NCH):
                        c0, c1 = ch * 512, (ch + 1) * 512
                        ps = psA()
                        mm(ps[:], [(BSQ[:, kt, DIM + t * P:DIM + (t + 1) * P],
                                    t1_fm[:, kt, c0:c1]) for kt in range(2)])
                        nc.scalar.copy(raw_fm[:, t, c0:c1], ps[:])
                if not out_is_tm:
                    for t in range(2):
                        for ch in range(NCH):
                            c0, c1 = ch * 512, (ch + 1) * 512
                            ps = psA()
                            mm(ps[:], [(BSQ[:, kt, 2 * DIM + t * P:
                                            2 * DIM + (t + 1) * P],
                                        raw_fm[:, kt, c0:c1])
                                       for kt in range(2)])
                            nc.vector.scalar_tensor_tensor(
                                out_tile[:, t, c0:c1], ps[:], V('catvec0', t),
                                res[:, t, c0:c1], OP.add, OP.add)
                else:
                    for mt in range(LT):
                        ps = psA()
                        mm(ps[:, :DIM], [(raw_fm[:, kt, mt * P:(mt + 1) * P],
                                          BSQ[:, kt, 2 * DIM:3 * DIM])
                                         for kt in range(2)])
                        tmp = mp.tile([P, DIM], F32, tag="cat_tmp",
                                      name="cat_tmp")
                        nc.vector.tensor_tensor(tmp[:], ps[:, :DIM],
                                                catvec1_bc, OP.add)
                        nc.vector.tensor_tensor(out_tile[:, mt, :], tmp[:],
                                                res[:, mt, :], OP.add)

        mamba(0, x0_fm, NTOK, x0_fm, out0_fm, False)
        mamba(1, x1_fm, MANCH, x1_tm, out1_tm, True)

        # ================= cross attention =================
        with ExitStack() as ces:
            cp_ = ces.enter_context(tc.tile_pool(name="cross", bufs=1))
            load_w('cbrw', cp_)
            load_w('unmap_T', cp_)
            CBO = {nm: i * DIM for i, nm in enumerate(_CBR)}
            out1_fm = cp_.tile([P, 2, MANCH], BF16, tag="out1_fm")
            to_fm(out1_tm, out1_fm, MT)

            def cbr_fm(nm, src_fm, L, dst):
                for t in range(2):
                    for ch in range(L // 512):
                        c0, c1 = ch * 512, (ch + 1) * 512
                        ps = psA()
                        mm(ps[:], [(W['cbrw'][:, kt, CBO[nm] + t * P:
                                              CBO[nm] + (t + 1) * P],
                                    src_fm[:, kt, c0:c1]) for kt in range(2)])
                        nc.scalar.activation(dst[:, t, c0:c1], ps[:], AT.Relu,
                                             scale=V(f'{nm}_s', t),
                                             bias=V(f'{nm}_t', t))

            av0_tm = cp_.tile([P, NT, DIM], BF16, tag="av0_tm")
            av1_tm = cp_.tile([P, MT, DIM], BF16, tag="av1_tm")
            with ExitStack() as aes:
                ap_ = aes.enter_context(tc.tile_pool(name="attn", bufs=1))
                q_bf = ap_.tile([P, 2, NTOK], BF16, tag="q")
                k_bf = ap_.tile([P, 2, MANCH], BF16, tag="k")
                v0_bf = ap_.tile([P, 2, MANCH], BF16, tag="v0")
                v1_bf = ap_.tile([P, 2, NTOK], BF16, tag="v1")
                cbr_fm('q', out0_fm, NTOK, q_bf)
                cbr_fm('k', out1_fm, MANCH, k_bf)
                cbr_fm('v0', out1_fm, MANCH, v0_bf)
                cbr_fm('v1', out0_fm, NTOK, v1_bf)
                v0_tm = ap_.tile([P, MT, DIM], BF16, tag="v0_tm")
                v1_tm = ap_.tile([P, NT, DIM], BF16, tag="v1_tm")
                to_tm(v0_bf, v0_tm, MT, bf=True)
                to_tm(v1_bf, v1_tm, NT, bf=True)

                for h in range(HEADS):
                    t, row = h // 2, (h % 2) * HD
                    E_h = ap_.tile([P, NT, MANCH], BF16, tag="E_h",
                                   name="E_h", bufs=2)
                    ET_h = ap_.tile([P, MT, NTOK], BF16, tag="ET_h",
                                    name="ET_h", bufs=2)
                    sm = ap_.tile([P, 2 * NT + 4 * MT], F32, tag="sm",
                                  name="sm", bufs=2)
                    rs0, r0att = sm[:, 0:NT], sm[:, NT:2 * NT]
                    cs_p = sm[:, 2 * NT:2 * NT + 2 * MT]
                    cs_tot = sm[:, 2 * NT + 2 * MT:2 * NT + 3 * MT]
                    r1att = sm[:, 2 * NT + 3 * MT:2 * NT + 4 * MT]
                    for i in range(NT):
                        ps = psA()
                        mm(ps[:], [(q_bf[row:row + HD, t, i * P:(i + 1) * P],
                                    k_bf[row:row + HD, t, :])], bf=True)
                        nc.scalar.activation(E_h[:, i, :], ps[:], AT.Exp,
                                             accum_out=rs0[:, i:i + 1])
                    nc.vector.reciprocal(r0att, rs0)
                    for mt in range(MT):
                        for i0 in range(0, NT, 4):
                            ps = psT(BF16)
                            for j in range(4):
                                i = i0 + j
                                nc.tensor.transpose(
                                    ps[:, j * P:(j + 1) * P],
                                    E_h[:, i, mt * P:(mt + 1) * P], IDb[:])
                            nc.scalar.activation(
                                ET_h[:, mt, i0 * P:(i0 + 4) * P],
                                ps[:, :4 * P], AT.Copy,
                                accum_out=cs_p[:, mt * 2 + i0 // 4:
                                               mt * 2 + i0 // 4 + 1])
                    nc.vector.reduce_sum(
                        cs_tot, cs_p.rearrange("p (m c) -> p m c", c=2),
                        axis=AX.X)
                    nc.vector.reciprocal(r1att, cs_tot)
                    for i in range(NT):
                        ps = pY.tile([P, P], F32, tag="ps", name="psy")
                        for mt in range(MT):
                            nc.tensor.matmul(
                                ps[:, :HD], ET_h[:, mt, i * P:(i + 1) * P],
                                v0_tm[:, mt, h * HD:(h + 1) * HD],
                                start=(mt == 0), stop=(mt == MT - 1))
                        nc.vector.tensor_scalar_mul(
                            av0_tm[:, i, h * HD:(h + 1) * HD],
                            ps[:, :HD], r0att[:, i:i + 1])
                    for mt in range(MT):
                        ps = pY.tile([P, P], F32, tag="ps", name="psy")
                        for i in range(NT):
                            nc.tensor.matmul(
                                ps[:, :HD], E_h[:, i, mt * P:(mt + 1) * P],
                                v1_tm[:, i, h * HD:(h + 1) * HD],
                                start=(i == 0), stop=(i == NT - 1))
                        nc.vector.tensor_scalar_mul(
                            av1_tm[:, mt, h * HD:(h + 1) * HD],
                            ps[:, :HD], r1att[:, mt:mt + 1])

            av0_fm = cp_.tile([P, 2, NTOK], BF16, tag="av0_fm")
            av1_fm = cp_.tile([P, 2, MANCH], BF16, tag="av1_fm")
            to_fm(av0_tm, av0_fm, NT)
            to_fm(av1_tm, av1_fm, MT)
            c0_fm = cp_.tile([P, 2, NTOK], BF16, tag="c0_fm")
            c1_fm = cp_.tile([P, 2, MANCH], BF16, tag="c1_fm")
            cbr_fm('mh0', av0_fm, NTOK, c0_fm)
            cbr_fm('mh1', av1_fm, MANCH, c1_fm)
            out0f_fm = cp_.tile([P, 2, NTOK], BF16, tag="av0_fm", name="out0f")
            out1f_fm = cp_.tile([P, 2, MANCH], BF16, tag="av1_fm", name="out1f")
            nc.vector.tensor_tensor(out0f_fm[:], out0_fm[:], c0_fm[:], OP.add)
            nc.vector.tensor_tensor(out1f_fm[:], out1_fm[:], c1_fm[:], OP.add)
            out1f_tm = cp_.tile([P, MT, DIM], BF16, tag="out1f_tm")
            to_tm(out1f_fm, out1f_tm, MT)

            # ---- unmap softmax + scatter ----
            E1_bf = cp_.tile([P, MT, NTOK], BF16, tag="E1")
            for mt in range(MT):
                for ch in range(2):
                    ps = psA()
                    mm(ps[:], [(W['unmap_T'][:, kt, mt * P:(mt + 1) * P],
                                x0_fm[:, kt, ch * 512:(ch + 1) * 512])
                               for kt in range(2)])
                    nc.scalar.activation(E1_bf[:, mt, ch * 512:(ch + 1) * 512],
                                         ps[:], AT.Exp)
            r1_row = cp_.tile([1, NTOK], F32, tag="r1_row")
            for ch in range(2):
                ps = psA()
                for mt in range(MT):
                    nc.tensor.matmul(ps[:1, :], W['ones_bf'][:],
                                     E1_bf[:, mt, ch * 512:(ch + 1) * 512],
                                     start=(mt == 0), stop=(mt == MT - 1))
                nc.vector.reciprocal(r1_row[:, ch * 512:(ch + 1) * 512],
                                     ps[:1, :])
            r1row_bf = cp_.tile([1, NTOK], BF16, tag="r1bf", name="r1bf")
            nc.vector.tensor_copy(r1row_bf[:], r1_row[:])
            nc.sync.dma_start(r1_stage, r1row_bf[:])
            r1_bc = cp_.tile([P, NTOK], BF16, tag="r1_bc")
            bcast_row(r1_bc[:], r1_stage)
            E1s_bf = cp_.tile([P, MT, NTOK], BF16, tag="E1s")
            for mt in range(MT):
                nc.vector.tensor_tensor(E1s_bf[:, mt, :], E1_bf[:, mt, :],
                                        r1_bc[:], OP.mult)
            scat_fm = cp_.tile([P, 2, NTOK], BF16, tag="c0_fm", name="scat_fm")
            for ct in range(2):
                for ch in range(2):
                    ps = psA()
                    mm(ps[:], [(out1f_tm[:, mt, ct * P:(ct + 1) * P],
                                E1s_bf[:, mt, ch * 512:(ch + 1) * 512])
                               for mt in range(MT)], bf=True)
                    nc.scalar.copy(scat_fm[:, ct, ch * 512:(ch + 1) * 512],
                                   ps[:])

            # ---- final projection + skip ----
            xb_tm = cp_.tile([P, NT, DIM], F32, tag="E1", name="xb_tm")
            nc.sync.dma_start(xb_tm[:], xb_d.rearrange("(i p) c -> p i c", p=P))
            out_sb = cp_.tile([P, NT, DIM], F32, tag="E1s", name="out_sb")
            for i in range(NT):
                ps = psA()
                pairs = [(out0f_fm[:, kt, i * P:(i + 1) * P],
                          W['bigsq0'][:, kt, 3 * DIM:4 * DIM])
                         for kt in range(2)]
                pairs += [(scat_fm[:, kt, i * P:(i + 1) * P],
                           W['bigsq0'][:, kt, 3 * DIM:4 * DIM])
                          for kt in range(2)]
                mm(ps[:, :DIM], pairs)
                nc.vector.tensor_tensor(out_sb[:, i, :], ps[:, :DIM],
                                        xb_tm[:, i, :], OP.add)
            nc.sync.dma_start(out_d.rearrange("(i p) c -> p i c", p=P),
                              out_sb[:])

    nc.compile()
    return nc


# ---------------------------------------------------------------- entry

def kernel(x, params):
    x = np.asarray(x, dtype=np.float32)
    params = _to_np(params)
    cm, consts = _prep(x, params)
    if 'prog' not in _prog_cache:
        _prog_cache['prog'] = _build(consts)
    nc = _prog_cache['prog']

    xb = (x + np.asarray(params['proj_b'], np.float32)).astype(np.float32)
    in_maps = []
    for c in range(NCORES):
        m = dict(cm)
        m['x'] = np.ascontiguousarray(x[c])
        m['xb'] = np.ascontiguousarray(xb[c])
        in_maps.append(m)

    import os
    from concourse import bass_utils
    res = bass_utils.run_bass_kernel_spmd(
        nc, in_maps, core_ids=list(range(NCORES)),
        trace=bool(os.environ.get('BASSK_TRACE')))
    global _last_result
    _last_result = res
    return np.stack([r['out'] for r in res.results], axis=0).astype(np.float32)


_last_result = None


def _to_np(obj):
    if isinstance(obj, dict):
        return {k: _to_np(v) for k, v in obj.items()}
    return np.asarray(obj)
